# revision 7
# baseline (speedup 1.0000x reference)
"""AttentionBlock (GroupNorm + 1x1-conv QKV self-attention + out-proj + residual)
as a distributed Bass kernel on 8 TRN2 NeuronCores.

Sharding: fully data-parallel, zero collectives.
  core = 2*b + half   (b in 0..3 batch element, half in 0..1 query-row half)
Each core:
  - GroupNorm stats+apply for its batch element (duplicated within a pair;
    cheaper than exchanging k/v via collectives)
  - k, vT projections over all 4096 tokens; q projection over its 2048 rows
  - attention in transposed layout: s_T[m, n] = sum_c k[c,m] q[c,n]
    -> exp on ScalarE -> PV accumulation with vT as stationary operand,
    softmax denominator via a fused ones-matmul, normalization deferred
    to after PV (scale by 1/Z), out-proj, +bias +residual.
  - No on-chip transposes needed anywhere (vT produced directly by using
    h as the stationary matmul operand; weights pre-transposed on host).

Matmul operands bf16 (psum accumulates fp32); GN/softmax stats fp32.
"""

import os
import sys

import numpy as np

for p in ("/opt/trn_rl_repo", "/opt/pypackages"):
    if p not in sys.path:
        sys.path.append(p)

import ml_dtypes

import concourse.bass as bass
import concourse.bacc as bacc
import concourse.tile as tile
from concourse import mybir
from concourse.bass import ts
from concourse.bass_utils import run_bass_kernel_spmd

F32 = mybir.dt.float32
BF16 = mybir.dt.bfloat16
AF = mybir.ActivationFunctionType
OP = mybir.AluOpType

C = 512
N = 4096
NHALF = 2048
P = 128
CCH = C // P          # 4 channel chunks
NB = N // 512         # 8 column blocks of 512
NBH = NHALF // 512    # 4
MC = N // P           # 32 key chunks of 128
EPS = 1e-5
SCALE = C ** -0.5

LAST_EXEC_TIME_NS = None

_CACHED_NC = None
_last_in_maps = None


def build_nc():
    nc = bacc.Bacc(None, target_bir_lowering=False)

    x_full = nc.declare_dram_parameter("x_full", [CCH, P, N], F32, isOutput=False)
    x_my = nc.declare_dram_parameter("x_my", [CCH, P, NHALF], F32, isOutput=False)
    wq_p = nc.declare_dram_parameter("wqT", [P, CCH, C], BF16, isOutput=False)
    wk_p = nc.declare_dram_parameter("wkT", [P, CCH, C], BF16, isOutput=False)
    wv_p = nc.declare_dram_parameter("wvT", [P, CCH, C], BF16, isOutput=False)
    wo_p = nc.declare_dram_parameter("woT", [P, CCH, C], BF16, isOutput=False)
    bq_p = nc.declare_dram_parameter("bq", [P, CCH], F32, isOutput=False)
    bk_p = nc.declare_dram_parameter("bk", [P, CCH], F32, isOutput=False)
    bo_p = nc.declare_dram_parameter("bo", [P, CCH], F32, isOutput=False)
    bv_p = nc.declare_dram_parameter("bv_rep", [P, C], F32, isOutput=False)
    gnw_p = nc.declare_dram_parameter("gnw", [P, CCH], F32, isOutput=False)
    gnb_p = nc.declare_dram_parameter("gnb", [P, CCH], F32, isOutput=False)
    ones_p = nc.declare_dram_parameter("ones", [P, P], BF16, isOutput=False)
    ind_p = nc.declare_dram_parameter("ind", [P, 8], F32, isOutput=False)
    ind2_p = nc.declare_dram_parameter("ind2", [8, P], F32, isOutput=False)
    out_p = nc.declare_dram_parameter("out", [CCH, P, NHALF], F32, isOutput=True)

    with tile.TileContext(nc) as tc:
        with tc.tile_pool(name="singles", bufs=1) as singles:
            k_t = singles.tile([P, CCH, N], BF16)
            q_t = singles.tile([P, CCH, NHALF], BF16)
            vT_t = singles.tile([P, MC, C], BF16)
            xb_t = singles.tile([P, CCH, NHALF], F32)
            A_t = singles.tile([P, CCH], F32)
            B_t = singles.tile([P, CCH], F32)
            w_q = singles.tile([P, CCH, C], BF16)
            w_k = singles.tile([P, CCH, C], BF16)
            w_v = singles.tile([P, CCH, C], BF16)
            w_o = singles.tile([P, CCH, C], BF16)
            bq_t = singles.tile([P, CCH], F32)
            bk_t = singles.tile([P, CCH], F32)
            bo_t = singles.tile([P, CCH], F32)
            bv_t = singles.tile([P, C], F32)
            gnw_t = singles.tile([P, CCH], F32)
            gnb_t = singles.tile([P, CCH], F32)
            ones_t = singles.tile([P, P], BF16)
            ind_t = singles.tile([P, 8], F32)
            ind2_t = singles.tile([8, P], F32)
            eps_t = singles.tile([P, 1], F32)
            zero_t = singles.tile([P, 1], F32)
            nc.vector.memset(eps_t, EPS)
            nc.vector.memset(zero_t, 0.0)

            nc.sync.dma_start(out=w_q, in_=wq_p[:])
            nc.sync.dma_start(out=w_k, in_=wk_p[:])
            nc.sync.dma_start(out=w_v, in_=wv_p[:])
            nc.sync.dma_start(out=w_o, in_=wo_p[:])
            nc.sync.dma_start(out=bq_t, in_=bq_p[:])
            nc.sync.dma_start(out=bk_t, in_=bk_p[:])
            nc.sync.dma_start(out=bo_t, in_=bo_p[:])
            nc.sync.dma_start(out=bv_t, in_=bv_p[:])
            nc.sync.dma_start(out=gnw_t, in_=gnw_p[:])
            nc.sync.dma_start(out=gnb_t, in_=gnb_p[:])
            nc.sync.dma_start(out=ones_t, in_=ones_p[:])
            nc.sync.dma_start(out=ind_t, in_=ind_p[:])
            nc.sync.dma_start(out=ind2_t, in_=ind2_p[:])

            # ---------------- Phase A: GroupNorm statistics ----------------
            # Per channel-chunk: per-row mean/var over n via bn_stats/bn_aggr,
            # then combine across the 16 rows of each group with two tiny
            # matmuls (indicator matrices), giving per-row affine A, B with
            # h = x*A + B.
            with (
                tc.tile_pool(name="astat", bufs=4) as statp,
                tc.tile_pool(name="aload", bufs=4) as aload,
                tc.tile_pool(name="apsum", bufs=2, space="PSUM") as app,
            ):
                for ci in range(CCH):
                    st6 = statp.tile([P, NB, 6], F32, tag="st6")
                    for nb in range(NB):
                        xt = aload.tile([P, 512], F32, tag="xt")
                        nc.sync.dma_start(out=xt, in_=x_full[ci, :, ts(nb, 512)])
                        nc.vector.bn_stats(out=st6[:, nb, :], in_=xt)
                    mv = statp.tile([P, 2], F32, tag="mv")
                    nc.vector.bn_aggr(out=mv, in_=st6)
                    rs = statp.tile([P, 2], F32, tag="rs")
                    nc.vector.tensor_mul(out=rs[:, 1:2], in0=mv[:, 0:1], in1=mv[:, 0:1])
                    nc.vector.tensor_add(out=rs[:, 1:2], in0=rs[:, 1:2], in1=mv[:, 1:2])
                    nc.vector.tensor_copy(out=rs[:, 0:1], in_=mv[:, 0:1])
                    gps = app.tile([8, 2], F32, tag="g")
                    nc.tensor.matmul(gps, lhsT=ind_t, rhs=rs, start=True, stop=True)
                    gsb = statp.tile([8, 2], F32, tag="gsb")
                    nc.vector.tensor_copy(out=gsb, in_=gps)
                    rps = app.tile([P, 2], F32, tag="r")
                    nc.tensor.matmul(rps, lhsT=ind2_t, rhs=gsb, start=True, stop=True)
                    gm = statp.tile([P, 1], F32, tag="gm")
                    gv = statp.tile([P, 1], F32, tag="gv")
                    nc.vector.tensor_copy(out=gm, in_=rps[:, 0:1])
                    nc.vector.tensor_mul(out=gv, in0=gm, in1=gm)
                    nc.vector.tensor_sub(out=gv, in0=rps[:, 1:2], in1=gv)
                    # rstd = 1/sqrt(var + eps)
                    nc.scalar.activation(out=gv, in_=gv, func=AF.Sqrt, bias=eps_t)
                    nc.vector.reciprocal(out=gv, in_=gv)
                    nc.vector.tensor_mul(
                        out=A_t[:, ci : ci + 1], in0=gv, in1=gnw_t[:, ci : ci + 1]
                    )
                    nc.vector.tensor_mul(out=gm, in0=gm, in1=A_t[:, ci : ci + 1])
                    nc.vector.tensor_sub(
                        out=B_t[:, ci : ci + 1], in0=gnb_t[:, ci : ci + 1], in1=gm
                    )

            # ---------------- Phase B: projections ----------------
            with (
                tc.tile_pool(name="hx", bufs=3) as hxp,
                tc.tile_pool(name="bpsum", bufs=2, space="PSUM") as bpp,
            ):
                for nb in range(NB):
                    hb = hxp.tile([P, CCH, 512], BF16, tag="hb")
                    for ci in range(CCH):
                        xt = hxp.tile([P, 512], F32, tag="xt")
                        nc.sync.dma_start(out=xt, in_=x_full[ci, :, ts(nb, 512)])
                        nc.vector.tensor_scalar(
                            out=hb[:, ci, :],
                            in0=xt,
                            scalar1=A_t[:, ci : ci + 1],
                            scalar2=B_t[:, ci : ci + 1],
                            op0=OP.mult,
                            op1=OP.add,
                        )
                    for oj in range(CCH):
                        kp = bpp.tile([P, 512], F32, tag="pj")
                        for ci in range(CCH):
                            nc.tensor.matmul(
                                kp,
                                lhsT=w_k[:, ci, ts(oj, P)],
                                rhs=hb[:, ci, :],
                                start=(ci == 0),
                                stop=(ci == CCH - 1),
                            )
                        nc.vector.tensor_scalar_add(
                            out=k_t[:, oj, ts(nb, 512)],
                            in0=kp,
                            scalar1=bk_t[:, oj : oj + 1],
                        )
                    for mj in range(4):
                        vp = bpp.tile([P, 512], F32, tag="pj")
                        for ci in range(CCH):
                            nc.tensor.matmul(
                                vp,
                                lhsT=hb[:, ci, ts(mj, P)],
                                rhs=w_v[:, ci, :],
                                start=(ci == 0),
                                stop=(ci == CCH - 1),
                            )
                        nc.vector.tensor_add(
                            out=vT_t[:, nb * 4 + mj, :], in0=vp, in1=bv_t
                        )
                for nb in range(NBH):
                    hb = hxp.tile([P, CCH, 512], BF16, tag="hb")
                    for ci in range(CCH):
                        xt = hxp.tile([P, 512], F32, tag="xt")
                        nc.sync.dma_start(out=xt, in_=x_my[ci, :, ts(nb, 512)])
                        # xb = x + bo (for the final residual+bias pass)
                        nc.vector.tensor_scalar_add(
                            out=xb_t[:, ci, ts(nb, 512)],
                            in0=xt,
                            scalar1=bo_t[:, ci : ci + 1],
                        )
                        nc.vector.tensor_scalar(
                            out=hb[:, ci, :],
                            in0=xt,
                            scalar1=A_t[:, ci : ci + 1],
                            scalar2=B_t[:, ci : ci + 1],
                            op0=OP.mult,
                            op1=OP.add,
                        )
                    for oj in range(CCH):
                        qp = bpp.tile([P, 512], F32, tag="pj")
                        for ci in range(CCH):
                            nc.tensor.matmul(
                                qp,
                                lhsT=w_q[:, ci, ts(oj, P)],
                                rhs=hb[:, ci, :],
                                start=(ci == 0),
                                stop=(ci == CCH - 1),
                            )
                        nc.vector.tensor_scalar_add(
                            out=q_t[:, oj, ts(nb, 512)],
                            in0=qp,
                            scalar1=bq_t[:, oj : oj + 1],
                        )

            # ---------------- Phase C: attention + out-proj + residual ----
            with (
                tc.tile_pool(name="sps", bufs=2, space="PSUM") as sps,
                tc.tile_pool(name="ozp", bufs=5, space="PSUM") as ozp,
                tc.tile_pool(name="att", bufs=3) as attp,
                tc.tile_pool(name="fin", bufs=2) as finp,
            ):
                for ns in range(NBH):
                    ops = [
                        ozp.tile([P, 512], F32, tag="oz", name=f"o{ns}_{e4}")
                        for e4 in range(CCH)
                    ]
                    zps = ozp.tile([P, 512], F32, tag="oz")
                    # software-pipelined: s(mc+1) overlaps exp(mc) on ScalarE
                    sp_tiles = [None] * MC

                    def emit_s(mc):
                        sp = sps.tile([P, 512], F32, tag="s")
                        for ci in range(CCH):
                            nc.tensor.matmul(
                                sp,
                                lhsT=k_t[:, ci, ts(mc, P)],
                                rhs=q_t[:, ci, ts(ns, 512)],
                                start=(ci == 0),
                                stop=(ci == CCH - 1),
                            )
                        sp_tiles[mc] = sp

                    emit_s(0)
                    for mc in range(MC):
                        if mc + 1 < MC:
                            emit_s(mc + 1)
                        et = attp.tile([P, 512], BF16, tag="e")
                        nc.scalar.activation(
                            out=et,
                            in_=sp_tiles[mc],
                            func=AF.Exp,
                            bias=zero_t,
                            scale=SCALE,
                        )
                        sp_tiles[mc] = None
                        for e4 in range(CCH):
                            nc.tensor.matmul(
                                ops[e4],
                                lhsT=vT_t[:, mc, ts(e4, P)],
                                rhs=et,
                                start=(mc == 0),
                                stop=(mc == MC - 1),
                            )
                        nc.tensor.matmul(
                            zps,
                            lhsT=ones_t,
                            rhs=et,
                            start=(mc == 0),
                            stop=(mc == MC - 1),
                        )
                    rz = attp.tile([P, 512], F32, tag="rz")
                    nc.vector.reciprocal(out=rz, in_=zps)
                    osb = attp.tile([P, CCH, 512], BF16, tag="osb")
                    for e4 in range(CCH):
                        nc.vector.tensor_mul(out=osb[:, e4, :], in0=ops[e4], in1=rz)
                    for oj in range(CCH):
                        pp = sps.tile([P, 512], F32, tag="s")
                        for e4 in range(CCH):
                            nc.tensor.matmul(
                                pp,
                                lhsT=w_o[:, e4, ts(oj, P)],
                                rhs=osb[:, e4, :],
                                start=(e4 == 0),
                                stop=(e4 == CCH - 1),
                            )
                        res = finp.tile([P, 512], F32, tag="res")
                        nc.vector.tensor_add(
                            out=res, in0=pp, in1=xb_t[:, oj, ts(ns, 512)]
                        )
                        nc.sync.dma_start(out=out_p[oj, :, ts(ns, 512)], in_=res)

    nc.compile()
    return nc


def _prep_consts(inputs):
    bf = ml_dtypes.bfloat16

    def wt(w):
        # w: [o, c] -> lhsT layout [c, o] chunked by c: [P, CCH, C]
        return np.ascontiguousarray(
            w.T.reshape(CCH, P, C).transpose(1, 0, 2)
        ).astype(bf)

    def colvec(b):
        return np.ascontiguousarray(b.reshape(CCH, P).T).astype(np.float32)

    ind = np.zeros((P, 8), np.float32)
    ind[np.arange(P), np.arange(P) // 16] = 1.0 / 16.0
    ind2 = np.zeros((8, P), np.float32)
    ind2[np.arange(P) // 16, np.arange(P)] = 1.0

    return {
        "wqT": wt(np.asarray(inputs["wq"], np.float32)),
        "wkT": wt(np.asarray(inputs["wk"], np.float32)),
        "wvT": wt(np.asarray(inputs["wv"], np.float32)),
        "woT": wt(np.asarray(inputs["wo"], np.float32)),
        "bq": colvec(np.asarray(inputs["bq"], np.float32)),
        "bk": colvec(np.asarray(inputs["bk"], np.float32)),
        "bo": colvec(np.asarray(inputs["bo"], np.float32)),
        "bv_rep": np.ascontiguousarray(
            np.broadcast_to(np.asarray(inputs["bv"], np.float32), (P, C))
        ),
        "gnw": colvec(np.asarray(inputs["gn_w"], np.float32)),
        "gnb": colvec(np.asarray(inputs["gn_b"], np.float32)),
        "ones": np.ones((P, P), bf),
        "ind": ind,
        "ind2": ind2,
    }


def kernel(**inputs):
    global LAST_EXEC_TIME_NS, _CACHED_NC
    x = np.asarray(inputs["x"], np.float32)  # [4, 512, 64, 64]
    B = x.shape[0]
    assert x.shape == (4, C, 64, 64)

    if _CACHED_NC is None:
        _CACHED_NC = build_nc()
    nc = _CACHED_NC

    consts = _prep_consts(inputs)
    xf = np.ascontiguousarray(x.reshape(B, CCH, P, N))

    in_maps = []
    for core in range(8):
        b, half = core // 2, core % 2
        m = dict(consts)
        m["x_full"] = xf[b]
        m["x_my"] = np.ascontiguousarray(
            xf[b][:, :, half * NHALF : (half + 1) * NHALF]
        )
        in_maps.append(m)

    global _last_in_maps
    _last_in_maps = in_maps
    res = run_bass_kernel_spmd(nc, in_maps, core_ids=list(range(8)))
    LAST_EXEC_TIME_NS = res.exec_time_ns

    out = np.empty((B, C, N), np.float32)
    for core in range(8):
        b, half = core // 2, core % 2
        out[b, :, half * NHALF : (half + 1) * NHALF] = (
            res.results[core]["out"].reshape(C, NHALF)
        )
    return out.reshape(B, C, 64, 64)
